# revision 30
# baseline (speedup 1.0000x reference)
"""Autoregressive LSTM cell (B=256, T=256, D=256, H=1024, O=256) on 8 TRN2 cores.

Strategy: pure data-parallel over batch (32 rows/core, no collectives).
Per step t (sequential, 256 steps):
    z = x_t @ Wxx + y_{t-1} @ Wxy + h_{t-1} @ Wh     (+b)
    i,f,g,o gates -> c = sig(f)*c + sig(i)*tanh(g); h = sig(o)*tanh(c)
    y = tanh(h @ Wd + bd)
Matmuls are "activation-stationary": lhsT = activation^T [K<=128, M=32batch],
rhs streams bf16 weight columns at 1 column/cycle (fp32 PSUM accumulation;
fp32 gate math keeps the recurrence error ~1e-2-bounded).
4-way PE column tiling (tile_position col groups) packs 4 batch-32 matmuls
concurrently, writing z in a stacked PSUM layout:
  z_ps [128, 1024]: position (32j+b, 512*beta + n) = z_perm[2048*beta + 512j + n]
Weight columns are host-permuted so that
  bank0 (cols 0:512)  = [ sig-gate i | sig-gate f ] column-paired per channel
  bank1 (cols 512:1024) = [ tanh-gate g | sig-gate o ]
with channel(p=32j+b, n) = 256j + (n mod 256): all gate elementwise ops are
partition-aligned and the c update is a free-dim-shifted add.
h/y are fed back transposed via PE transpose-mode matmuls.

This runtime is per-instruction-dispatch-bound (~30-70 us per engine
instruction regardless of engine overlap; measured with single-engine
dependent/independent chains), so the kernel minimizes INSTRUCTION COUNT:
gate math runs as full-width ops (4 ACT + 4 DVE + 1 tanh-c per step instead
of the half-split 2x chain), the hT/yT feedback shares one [128,320] tile
with two copies, and x-loads / ys-stores are batched XB=8 steps per DMA.
The z matmul count (12 K-chunks x 8 psum-bank N-chunks = 96/step) is the
floor for batch-sharded f32 PSUM (one matmul output <= one 2KB bank).

Host<->device traffic (the other half of the wall-clock, ~30-70 MB/s axon
tunnel) is minimized: weights are uploaded as 1/8 row-shards per core and
AllGathered device-side (HBM->HBM collective) instead of 8x-replicated from
the host; x and both weight shards are packed into a SINGLE input tensor
per core (each separate transfer buffer pays ~60-80ms fixed tunnel cost);
and ys is returned as offset-uint8 q=rne(127y+127.5) (decoded (q-127.5)/127
on host; max err 3.94e-3 inside the 2e-2 budget).

Measured end-to-end error vs the fp32 jax reference: scale-relative absmax
1.11e-2 (bit-matches the numpy simulation of the kernel's quantization
chain). Cached re-run wall-clock: ~3.96s vs 5.1s for the replicated-weights
fp32-output baseline measured under the same tunnel conditions (8.07s as
originally quoted). Emitted-instruction audit: 233 BIR instructions/step
(108 matmult + 108 ldweights + 17 ACT/DVE/DMA) at ~40us dispatch each —
the matmult count is at the K(12) x psum-bank(8) tiling floor.
"""

import sys

for p in ("/opt/trn_rl_repo",):
    if p not in sys.path:
        sys.path.insert(0, p)

from contextlib import ExitStack

import numpy as np

import concourse.bacc as bacc
import concourse.bass as bass
import concourse.mybir as mybir
import concourse.tile as tile
from concourse.masks import make_identity

F32 = mybir.dt.float32
U8 = mybir.dt.uint8
AF = mybir.ActivationFunctionType

B, T, D, H, O = 256, 256, 256, 1024, 256
NCORES = 8
BL = B // NCORES  # 32
G4 = 4 * H  # 4096
KX, KY, KH = D // 128, O // 128, H // 128  # 2, 2, 8


def gate_perm() -> np.ndarray:
    """Map stored z column position -> original gate column (i,f,g,o order)."""
    perm = np.empty(G4, dtype=np.int64)
    for beta in (0, 1):
        for j in range(4):
            for half in (0, 1):
                gate = (0, 1, 2, 3)[2 * beta + half]
                src = 1024 * gate + 256 * j
                pos = 2048 * beta + 512 * j + 256 * half
                perm[pos : pos + 256] = np.arange(src, src + 256)
    return perm


def _hT_off(c: int) -> int:
    """Column offset of h^T chunk c (channels 128c:128c+128) inside hT_sb."""
    return 128 * (c % 2) + 32 * (c // 2)


def round_f32r(a: np.ndarray) -> np.ndarray:
    """Round-to-nearest-even fp32 -> fp32r (low 12 mantissa bits zeroed)."""
    u = np.ascontiguousarray(a, dtype=np.float32).view(np.uint32)
    lsb = (u >> np.uint32(12)) & np.uint32(1)
    u = (u + np.uint32(0x7FF) + lsb) & np.uint32(0xFFFFF000)
    return u.view(np.float32)


FUNNEL = False


WROWS = D + O + H  # 1536 rows of the [*, 4H] weight blob (Wxx | Wxy | Wh)
WSH = WROWS // NCORES  # 192
WDSH = H // NCORES  # 128
WFS = WSH * G4 // (128 * 2 * BL)  # 96 fake steps carrying the wsh bytes
DFS = WDSH * O // (128 * 2 * BL)  # 4 fake steps carrying the wdsh bytes


XB = 8  # x-load / ys-store DMA batching (steps per DMA)


def build_nc(T_steps: int = T, use_bias_z: bool = False, use_bias_y: bool = False,
             mm_dt=mybir.dt.bfloat16, steps_exec: int | None = None):
    if steps_exec is None:
        steps_exec = T_steps
    nc = bacc.Bacc(num_devices=NCORES)

    # one packed input tensor: x steps 0:T, then the 1/8 weight-blob row-
    # shard (WFS fake steps) and the Wd row-shard (DFS fake steps); weight
    # shards are AllGathered device-side (the host->device tunnel is the
    # wall-clock bottleneck, NeuronLink is not)
    xb_d = nc.declare_dram_parameter("xb", [T_steps + WFS + DFS, 128, 2 * BL],
                                     mm_dt, isOutput=False)
    bz_d = by_d = None
    if use_bias_z:
        bz_d = nc.declare_dram_parameter("bz", [128, 1024], F32, isOutput=False)
    if use_bias_y:
        by_d = nc.declare_dram_parameter("by", [BL, O], F32, isOutput=False)
    # ys returned as offset-uint8: q = rne(127*y + 127.5), decoded on host as
    # (q - 127.5)/127 (max err 3.94e-3, well inside the 2e-2 budget). Halves
    # the D2H bytes vs bf16 and quarters them vs f32.
    ys_d = nc.declare_dram_parameter("ys", [BL, T_steps, O], U8, isOutput=True)

    def mc(ap):
        return ap.bitcast(mm_dt) if ap.dtype != mm_dt else ap

    with tile.TileContext(nc) as tc:
        with ExitStack() as ctx:
            wpool = ctx.enter_context(tc.tile_pool(name="weights", bufs=1))
            state = ctx.enter_context(tc.tile_pool(name="state", bufs=1))
            xpool = ctx.enter_context(tc.tile_pool(name="xin", bufs=3))
            gpool = ctx.enter_context(tc.tile_pool(name="gates", bufs=1))
            hpool = ctx.enter_context(tc.tile_pool(name="hT", bufs=1))
            ypool = ctx.enter_context(tc.tile_pool(name="yt", bufs=1))
            zpsum = ctx.enter_context(tc.tile_pool(name="zps", bufs=2, space="PSUM"))
            ypsum = ctx.enter_context(tc.tile_pool(name="yps", bufs=2, space="PSUM"))
            tpsum = ctx.enter_context(tc.tile_pool(name="tps", bufs=2, space="PSUM"))

            # device-side weight AllGather: each core contributes its 1/8
            # row-shard; afterwards every core has the full blob in HBM.
            dramw = ctx.enter_context(tc.tile_pool(name="dramw", bufs=1,
                                                   space="DRAM"))
            w_in = dramw.tile([WSH, G4], mm_dt, name="w_in")
            w_full = dramw.tile([WROWS, G4], mm_dt, name="w_full")
            wd_in = dramw.tile([WDSH, O], mm_dt, name="wd_in")
            wd_full = dramw.tile([H, O], mm_dt, name="wd_full")
            nc.gpsimd.dma_start(w_in[:], xb_d[T_steps : T_steps + WFS])
            nc.gpsimd.dma_start(wd_in[:], xb_d[T_steps + WFS : T_steps + WFS + DFS])
            RG = [list(range(NCORES))]
            nc.gpsimd.collective_compute(
                "AllGather", mybir.AluOpType.bypass, replica_groups=RG,
                ins=[w_in[:].opt()], outs=[w_full[:].opt()])
            nc.gpsimd.collective_compute(
                "AllGather", mybir.AluOpType.bypass, replica_groups=RG,
                ins=[wd_in[:].opt()], outs=[wd_full[:].opt()])

            Wxx_sb = wpool.tile([128, KX * G4], mm_dt)
            Wxy_sb = wpool.tile([128, KY * G4], mm_dt)
            Wh_sb = wpool.tile([128, KH * G4], mm_dt)
            Wd_sb = wpool.tile([128, KH * O], mm_dt)
            # Matmult instructions can carry at most ONE sem wait in this
            # lowering; every matmul dependency must resolve to a single DVE
            # sem value. Weight DMAs are therefore "laundered" through
            # in-place DVE copies (one per DMA so each copy waits on one
            # DMA-queue sem only).
            for k in range(KX):
                nc.sync.dma_start(Wxx_sb[:, k * G4 : (k + 1) * G4],
                                  w_full[k * 128 : (k + 1) * 128, :])
                nc.vector.tensor_copy(Wxx_sb[:, k * G4 : (k + 1) * G4],
                                      Wxx_sb[:, k * G4 : (k + 1) * G4])
            for k in range(KY):
                nc.sync.dma_start(Wxy_sb[:, k * G4 : (k + 1) * G4],
                                  w_full[D + k * 128 : D + (k + 1) * 128, :])
                nc.vector.tensor_copy(Wxy_sb[:, k * G4 : (k + 1) * G4],
                                      Wxy_sb[:, k * G4 : (k + 1) * G4])
            for k in range(KH):
                nc.sync.dma_start(Wh_sb[:, k * G4 : (k + 1) * G4],
                                  w_full[D + O + k * 128 : D + O + (k + 1) * 128, :])
                nc.vector.tensor_copy(Wh_sb[:, k * G4 : (k + 1) * G4],
                                      Wh_sb[:, k * G4 : (k + 1) * G4])
                nc.sync.dma_start(Wd_sb[:, k * O : (k + 1) * O],
                                  wd_full[k * 128 : (k + 1) * 128, :])
                nc.vector.tensor_copy(Wd_sb[:, k * O : (k + 1) * O],
                                      Wd_sb[:, k * O : (k + 1) * O])
            if use_bias_z:
                bz_sb = wpool.tile([128, 1024], F32)
                nc.sync.dma_start(bz_sb[:], bz_d[:, :])
            if use_bias_y:
                by_sb = wpool.tile([BL, O], F32)
                nc.sync.dma_start(by_sb[:], by_d[:, :])

            # identity for PE transposes (bf16: f32 transpose-mode faults on
            # hw); I64 in both partition halves so the fmap can start at
            # partition 0 or 64 (must match the weights)
            ident = wpool.tile([128, 128], mm_dt)
            make_identity(nc, ident[:])
            nc.vector.tensor_copy(ident[:], ident[:])  # launder Pool dep -> DVE

            # c state, channel(32j+b, n) = 256j + n
            c_sb = state.tile([128, 256], F32)
            nc.gpsimd.memset(c_sb[:], 0.0)

            def emit_z_mms(z_tile, chunks, start, stop):
                nck = len(chunks)
                for ci, (lhsT, wtile, coff) in enumerate(chunks):
                    for beta in range(2):
                        for j in range(4):
                            w_lo = coff + 2048 * beta + 512 * j
                            nc.tensor.matmul(
                                z_tile[32 * j : 32 * (j + 1),
                                       512 * beta : 512 * (beta + 1)],
                                mc(lhsT),
                                mc(wtile[:, w_lo : w_lo + 512]),
                                start=(start and ci == 0),
                                stop=(stop and ci == nck - 1),
                                tile_position=(0, 32 * j),
                                skip_group_check=True,
                            )

            # x loaded XB steps per DMA (per-instruction dispatch dominates
            # this runtime, so batch I/O into few big transfers)
            xblk = {}

            def x_chunks(t):
                b0 = t // XB
                if b0 not in xblk:
                    xT_sb = xpool.tile([128, XB * 2 * BL], mm_dt, name="xT_sb")
                    nc.sync.dma_start(
                        xT_sb[:],
                        xb_d[b0 * XB : (b0 + 1) * XB].transpose([1, 0, 2]))
                    # launder the x DMA-queue sem into the DVE sem
                    xr_sb = xpool.tile([128, XB * 2 * BL], mm_dt, name="xr_sb")
                    nc.vector.tensor_copy(xr_sb[:], xT_sb[:])
                    xblk[b0] = xr_sb
                base = (t % XB) * 2 * BL
                return [(xblk[b0][:, base + k * BL : base + (k + 1) * BL],
                         Wxx_sb, k * G4) for k in range(KX)]

            hyT_prev = None
            q_blk = None
            # software pipeline: the x-part of step t+1 is issued during step
            # t (cost-neutral in this dispatch-bound runtime, but keeps the
            # matmul dependency structure simple).
            z_ps = zpsum.tile([128, 1024], F32, name="z_ps")
            emit_z_mms(z_ps, x_chunks(0), start=True, stop=(steps_exec == 1))
            for t in range(steps_exec):
                if t > 0:
                    chunks = [(hyT_prev[:, _hT_off(k) : _hT_off(k) + BL], Wh_sb,
                               k * G4) for k in range(KH)]
                    chunks += [(hyT_prev[:, 256 + k * BL : 256 + (k + 1) * BL],
                                Wxy_sb, k * G4) for k in range(KY)]
                    emit_z_mms(z_ps, chunks, start=False, stop=True)
                if t + 1 < steps_exec:
                    z_next = zpsum.tile([128, 1024], F32, name="z_ps")
                    emit_z_mms(z_next, x_chunks(t + 1), start=True, stop=False)
                else:
                    z_next = None

                # gate math, full-width ops (<=1 PSUM operand per DVE op)
                if use_bias_z:
                    nc.vector.tensor_add(z_ps[:, 0:512], z_ps[:, 0:512],
                                         bz_sb[:, 0:512])
                    nc.vector.tensor_add(z_ps[:, 512:1024], z_ps[:, 512:1024],
                                         bz_sb[:, 512:1024])
                tg_sb = gpool.tile([128, 256], F32, name="tg_sb")
                h_stk = gpool.tile([128, 256], mm_dt, name="h_stk")
                tr_ps = tpsum.tile([128, 320], mm_dt, name="tr_ps")
                nc.scalar.activation(tg_sb[:], z_ps[:, 512:768], AF.Tanh)
                nc.scalar.activation(z_ps[:, 0:512], z_ps[:, 0:512], AF.Sigmoid)
                nc.vector.tensor_mul(tg_sb[:], z_ps[:, 0:256], tg_sb[:])
                nc.vector.tensor_mul(c_sb[:], z_ps[:, 256:512], c_sb[:])
                nc.scalar.activation(z_ps[:, 768:1024], z_ps[:, 768:1024],
                                     AF.Sigmoid)
                nc.vector.tensor_add(c_sb[:], tg_sb[:], c_sb[:])
                nc.scalar.activation(tg_sb[:], c_sb[:], AF.Tanh)
                nc.vector.tensor_mul(h_stk[:], z_ps[:, 768:1024], tg_sb[:])
                nc.tensor.transpose(tr_ps[:, 0:128], h_stk[:, 0:128], ident[:])
                nc.tensor.transpose(tr_ps[:, 128:256], h_stk[:, 128:256],
                                    ident[:])
                # hT needed now for the y-matmuls: copy it out of PSUM; the
                # yT columns (256:320) are appended after the y feedback so
                # one [128, 320] tile carries both transposed states
                hyT_sb = hpool.tile([128, 320], mm_dt, name="hyT_sb", bufs=2)
                nc.vector.tensor_copy(hyT_sb[:, 0:256], tr_ps[:, 0:256])

                # y = tanh(h @ Wd + bd)
                y_ps = ypsum.tile([BL, O], F32, name="y_ps")
                if FUNNEL:
                    nc.vector.tensor_copy(y_ps[:], Wxx_sb[0:BL, 0:256])
                for k in range(KH):
                    nc.tensor.matmul(
                        y_ps[:],
                        mc(hyT_sb[:, _hT_off(k) : _hT_off(k) + BL]),
                        mc(Wd_sb[:, k * O : (k + 1) * O]),
                        start=(k == 0),
                        stop=(k == KH - 1),
                    )
                if use_bias_y:
                    nc.vector.tensor_add(y_ps[:], y_ps[:], by_sb[:])
                y_bf = ypool.tile([BL, O], mm_dt, name="y_bf", bufs=2)
                nc.scalar.activation(y_bf[:], y_ps[:], AF.Tanh)
                # offset-uint8 quantize on ACT (rne conversion); XB steps
                # buffered in SBUF then stored with one DMA
                if t % XB == 0:
                    q_blk = ypool.tile([BL, XB * O], U8, name="q_blk", bufs=2)
                nc.scalar.activation(q_blk[:, (t % XB) * O : (t % XB + 1) * O],
                                     y_bf[:], AF.Copy, scale=127.0, bias=127.5)
                if t % XB == XB - 1 or t == steps_exec - 1:
                    t0 = t - (t % XB)
                    nc.sync.dma_start(ys_d[:, t0 : t + 1, :],
                                      q_blk[:, 0 : (t % XB + 1) * O])
                # DVE-launder for the PE transposes (matmul deps must
                # resolve to a single DVE sem value)
                y_tr = ypool.tile([BL, O], mm_dt, name="y_tr")
                nc.vector.tensor_copy(y_tr[:], y_bf[:])

                # y -> yT via 2 PE transposes
                for q in range(2):
                    nc.tensor.transpose(
                        tr_ps[:, 256 + 32 * q : 256 + 32 * (q + 1)],
                        y_tr[0:BL, 128 * q : 128 * (q + 1)],
                        ident[0:32, 0:32],
                    )
                nc.vector.tensor_copy(hyT_sb[:, 256:320], tr_ps[:, 256:320])

                hyT_prev = hyT_sb
                z_ps = z_next

    nc.compile()
    return nc


def prep_inputs(x, Wx, Wh, b, Wd, bd, T_steps: int = T):
    """Host-side shard + relayout. Returns (in_maps, use_bias_z, use_bias_y)."""
    x = np.asarray(x, dtype=np.float32)[:, :T_steps, :]
    Wx = np.asarray(Wx, dtype=np.float32)
    Wh = np.asarray(Wh, dtype=np.float32)
    b = np.asarray(b, dtype=np.float32)
    Wd = np.asarray(Wd, dtype=np.float32)
    bd = np.asarray(bd, dtype=np.float32)

    import ml_dtypes

    perm = gate_perm()
    Wxp = np.ascontiguousarray(Wx[:, perm]).astype(ml_dtypes.bfloat16)
    Whp = np.ascontiguousarray(Wh[:, perm]).astype(ml_dtypes.bfloat16)
    # blob rows: Wxx (0:D) | Wxy (D:D+O) | Wh (D+O:WROWS) — each core ships
    # 1/8 of the rows, the kernel AllGathers the full blob device-side
    wblob = np.concatenate([Wxp, Whp], axis=0)  # [WROWS, 4H]
    Wd = Wd.astype(ml_dtypes.bfloat16)

    use_bias_z = bool(np.any(b))
    use_bias_y = bool(np.any(bd))
    shared = {}
    if use_bias_z:
        bp = b[perm]
        bz = np.empty((128, 1024), dtype=np.float32)
        for j in range(4):
            for beta in range(2):
                bz[32 * j : 32 * (j + 1), 512 * beta : 512 * (beta + 1)] = bp[
                    2048 * beta + 512 * j : 2048 * beta + 512 * j + 512][None, :]
        shared["bz"] = bz
    if use_bias_y:
        shared["by"] = np.broadcast_to(bd, (BL, O)).copy()

    in_maps = []
    for c in range(NCORES):
        xc = x[c * BL : (c + 1) * BL]                      # [BL, T, D]
        xT = xc.transpose(1, 2, 0)                         # [T, D, BL]
        xT = xT.reshape(T_steps, 2, 128, BL).transpose(0, 2, 1, 3)
        xT = np.ascontiguousarray(
            xT.reshape(T_steps, 128, 2 * BL)).astype(ml_dtypes.bfloat16)
        # single packed input tensor per core: each transfer buffer pays a
        # large fixed cost on the axon tunnel (0.5MB standalone moved at
        # ~7MB/s), so x + weight shards ship as one buffer. The weight
        # shards are raw row-major bytes reinterpreted as "fake steps".
        xb = np.concatenate([
            xT,
            wblob[c * WSH : (c + 1) * WSH].reshape(WFS, 128, 2 * BL),
            Wd[c * WDSH : (c + 1) * WDSH].reshape(DFS, 128, 2 * BL),
        ], axis=0)
        in_maps.append({"xb": np.ascontiguousarray(xb), **shared})
    return in_maps, use_bias_z, use_bias_y


def postprocess(res):
    """Gather per-core offset-uint8 ys and decode to fp32 [B, T, O]."""
    q = np.concatenate([res.results[c]["ys"] for c in range(NCORES)], axis=0)
    return (q.astype(np.float32) - 127.5) * (1.0 / 127.0)


def kernel(x, Wx, Wh, b, Wd, bd):
    from concourse.bass_utils import run_bass_kernel_spmd

    in_maps, ubz, uby = prep_inputs(x, Wx, Wh, b, Wd, bd, T)
    nc = build_nc(T, ubz, uby)
    res = run_bass_kernel_spmd(nc, in_maps, list(range(NCORES)))
    return postprocess(res)



# revision 32
# speedup vs baseline: 1.1799x; 1.1799x over previous
"""Autoregressive LSTM cell (B=256, T=256, D=256, H=1024, O=256) on 8 TRN2 cores.

Strategy: pure data-parallel over batch (32 rows/core, no collectives).
Per step t (sequential, 256 steps):
    z = x_t @ Wxx + y_{t-1} @ Wxy + h_{t-1} @ Wh     (+b)
    i,f,g,o gates -> c = sig(f)*c + sig(i)*tanh(g); h = sig(o)*tanh(c)
    y = tanh(h @ Wd + bd)
Matmuls are "activation-stationary": lhsT = activation^T [K<=128, M=32batch],
rhs streams bf16 weight columns at 1 column/cycle (fp32 PSUM accumulation;
fp32 gate math keeps the recurrence error ~1e-2-bounded).
4-way PE column tiling (tile_position col groups) packs 4 batch-32 matmuls
concurrently, writing z in a stacked PSUM layout:
  z_ps [128, 1024]: position (32j+b, 512*beta + n) = z_perm[2048*beta + 512j + n]
Weight columns are host-permuted so that
  bank0 (cols 0:512)  = [ sig-gate i | sig-gate f ] column-paired per channel
  bank1 (cols 512:1024) = [ tanh-gate g | sig-gate o ]
with channel(p=32j+b, n) = 256j + (n mod 256): all gate elementwise ops are
partition-aligned and the c update is a free-dim-shifted add.
h/y are fed back transposed via PE transpose-mode matmuls.

This runtime is per-instruction-dispatch-bound (~30-70 us per engine
instruction regardless of engine overlap; measured with single-engine
dependent/independent chains), so the kernel minimizes INSTRUCTION COUNT:
gate math runs as full-width ops (4 ACT + 4 DVE + 1 tanh-c per step instead
of the half-split 2x chain), the hT/yT feedback shares one [128,320] tile
with two copies, and x-loads / ys-stores are batched XB=8 steps per DMA.
The z matmul count (12 K-chunks x 8 psum-bank N-chunks = 96/step) is the
floor for batch-sharded f32 PSUM (one matmul output <= one 2KB bank).

Host<->device traffic (the other half of the wall-clock, ~30-70 MB/s axon
tunnel) is minimized: weights are uploaded as 1/8 row-shards per core and
AllGathered device-side (HBM->HBM collective) instead of 8x-replicated from
the host; x and both weight shards are packed into a SINGLE input tensor
per core (each separate transfer buffer pays ~60-80ms fixed tunnel cost);
and ys is returned as offset-uint8 q=rne(127y+127.5) (decoded (q-127.5)/127
on host; max err 3.94e-3 inside the 2e-2 budget).

Measured end-to-end error vs the fp32 jax reference: scale-relative absmax
1.11e-2 (bit-matches the numpy simulation of the kernel's quantization
chain). Cached re-run wall-clock: ~3.96s vs 5.1s for the replicated-weights
fp32-output baseline measured under the same tunnel conditions (8.07s as
originally quoted). Emitted-instruction audit: 233 BIR instructions/step
(108 matmult + 108 ldweights + 17 ACT/DVE/DMA) at ~40us dispatch each —
the matmult count is at the K(12) x psum-bank(8) tiling floor.
"""

import sys

for p in ("/opt/trn_rl_repo",):
    if p not in sys.path:
        sys.path.insert(0, p)

from contextlib import ExitStack

import numpy as np

import concourse.bacc as bacc
import concourse.bass as bass
import concourse.mybir as mybir
import concourse.tile as tile
from concourse.masks import make_identity

F32 = mybir.dt.float32
U8 = mybir.dt.uint8
AF = mybir.ActivationFunctionType

B, T, D, H, O = 256, 256, 256, 1024, 256
NCORES = 8
BL = B // NCORES  # 32
G4 = 4 * H  # 4096
KX, KY, KH = D // 128, O // 128, H // 128  # 2, 2, 8


def gate_perm() -> np.ndarray:
    """Map stored z column position -> original gate column (i,f,g,o order)."""
    perm = np.empty(G4, dtype=np.int64)
    for beta in (0, 1):
        for j in range(4):
            for half in (0, 1):
                gate = (0, 1, 2, 3)[2 * beta + half]
                src = 1024 * gate + 256 * j
                pos = 2048 * beta + 512 * j + 256 * half
                perm[pos : pos + 256] = np.arange(src, src + 256)
    return perm


def _hT_off(c: int) -> int:
    """Column offset of h^T chunk c (channels 128c:128c+128) inside hT_sb."""
    return 128 * (c % 2) + 32 * (c // 2)


def round_f32r(a: np.ndarray) -> np.ndarray:
    """Round-to-nearest-even fp32 -> fp32r (low 12 mantissa bits zeroed)."""
    u = np.ascontiguousarray(a, dtype=np.float32).view(np.uint32)
    lsb = (u >> np.uint32(12)) & np.uint32(1)
    u = (u + np.uint32(0x7FF) + lsb) & np.uint32(0xFFFFF000)
    return u.view(np.float32)


FUNNEL = False


WROWS = D + O + H  # 1536 rows of the [*, 4H] weight blob (Wxx | Wxy | Wh)
WSH = WROWS // NCORES  # 192
WDSH = H // NCORES  # 128
WFS = WSH * G4 // (128 * 2 * BL)  # 96 fake steps carrying the wsh bytes
DFS = WDSH * O // (128 * 2 * BL)  # 4 fake steps carrying the wdsh bytes


XB = 8  # x-load / ys-store DMA batching (steps per DMA)


def build_nc(T_steps: int = T, use_bias_z: bool = False, use_bias_y: bool = False,
             mm_dt=mybir.dt.bfloat16, steps_exec: int | None = None):
    if steps_exec is None:
        steps_exec = T_steps
    nc = bacc.Bacc(num_devices=NCORES)

    # one packed input tensor: x steps 0:T, then the 1/8 weight-blob row-
    # shard (WFS fake steps) and the Wd row-shard (DFS fake steps); weight
    # shards are AllGathered device-side (the host->device tunnel is the
    # wall-clock bottleneck, NeuronLink is not)
    xb_d = nc.declare_dram_parameter("xb", [T_steps + WFS + DFS, 128, 2 * BL],
                                     mm_dt, isOutput=False)
    bz_d = by_d = None
    if use_bias_z:
        bz_d = nc.declare_dram_parameter("bz", [128, 1024], F32, isOutput=False)
    if use_bias_y:
        by_d = nc.declare_dram_parameter("by", [BL, O], F32, isOutput=False)
    # ys returned as offset-uint8: q = rne(127*y + 127.5), decoded on host as
    # (q - 127.5)/127 (max err 3.94e-3, well inside the 2e-2 budget). Halves
    # the D2H bytes vs bf16 and quarters them vs f32.
    ys_d = nc.declare_dram_parameter("ys", [BL, T_steps, O], U8, isOutput=True)

    def mc(ap):
        return ap.bitcast(mm_dt) if ap.dtype != mm_dt else ap

    with tile.TileContext(nc) as tc:
        with ExitStack() as ctx:
            wpool = ctx.enter_context(tc.tile_pool(name="weights", bufs=1))
            state = ctx.enter_context(tc.tile_pool(name="state", bufs=1))
            xpool = ctx.enter_context(tc.tile_pool(name="xin", bufs=3))
            gpool = ctx.enter_context(tc.tile_pool(name="gates", bufs=1))
            hpool = ctx.enter_context(tc.tile_pool(name="hT", bufs=1))
            ypool = ctx.enter_context(tc.tile_pool(name="yt", bufs=1))
            zpsum = ctx.enter_context(tc.tile_pool(name="zps", bufs=2, space="PSUM"))
            ypsum = ctx.enter_context(tc.tile_pool(name="yps", bufs=2, space="PSUM"))
            tpsum = ctx.enter_context(tc.tile_pool(name="tps", bufs=2, space="PSUM"))

            # device-side weight AllGather: each core contributes its 1/8
            # row-shard; afterwards every core has the full blob in HBM.
            dramw = ctx.enter_context(tc.tile_pool(name="dramw", bufs=1,
                                                   space="DRAM"))
            w_in = dramw.tile([WSH, G4], mm_dt, name="w_in")
            w_full = dramw.tile([WROWS, G4], mm_dt, name="w_full")
            wd_in = dramw.tile([WDSH, O], mm_dt, name="wd_in")
            wd_full = dramw.tile([H, O], mm_dt, name="wd_full")
            nc.gpsimd.dma_start(w_in[:], xb_d[T_steps : T_steps + WFS])
            nc.gpsimd.dma_start(wd_in[:], xb_d[T_steps + WFS : T_steps + WFS + DFS])
            RG = [list(range(NCORES))]
            nc.gpsimd.collective_compute(
                "AllGather", mybir.AluOpType.bypass, replica_groups=RG,
                ins=[w_in[:].opt()], outs=[w_full[:].opt()])
            nc.gpsimd.collective_compute(
                "AllGather", mybir.AluOpType.bypass, replica_groups=RG,
                ins=[wd_in[:].opt()], outs=[wd_full[:].opt()])

            Wxx_sb = wpool.tile([128, KX * G4], mm_dt)
            Wxy_sb = wpool.tile([128, KY * G4], mm_dt)
            Wh_sb = wpool.tile([128, KH * G4], mm_dt)
            Wd_sb = wpool.tile([128, KH * O], mm_dt)
            # Matmult instructions can carry at most ONE sem wait in this
            # lowering; every matmul dependency must resolve to a single DVE
            # sem value. Weight DMAs are therefore "laundered" through
            # in-place DVE copies (one per DMA so each copy waits on one
            # DMA-queue sem only).
            for k in range(KX):
                nc.sync.dma_start(Wxx_sb[:, k * G4 : (k + 1) * G4],
                                  w_full[k * 128 : (k + 1) * 128, :])
                nc.vector.tensor_copy(Wxx_sb[:, k * G4 : (k + 1) * G4],
                                      Wxx_sb[:, k * G4 : (k + 1) * G4])
            for k in range(KY):
                nc.sync.dma_start(Wxy_sb[:, k * G4 : (k + 1) * G4],
                                  w_full[D + k * 128 : D + (k + 1) * 128, :])
                nc.vector.tensor_copy(Wxy_sb[:, k * G4 : (k + 1) * G4],
                                      Wxy_sb[:, k * G4 : (k + 1) * G4])
            for k in range(KH):
                nc.sync.dma_start(Wh_sb[:, k * G4 : (k + 1) * G4],
                                  w_full[D + O + k * 128 : D + O + (k + 1) * 128, :])
                nc.vector.tensor_copy(Wh_sb[:, k * G4 : (k + 1) * G4],
                                      Wh_sb[:, k * G4 : (k + 1) * G4])
                nc.sync.dma_start(Wd_sb[:, k * O : (k + 1) * O],
                                  wd_full[k * 128 : (k + 1) * 128, :])
                nc.vector.tensor_copy(Wd_sb[:, k * O : (k + 1) * O],
                                      Wd_sb[:, k * O : (k + 1) * O])
            if use_bias_z:
                bz_sb = wpool.tile([128, 1024], F32)
                nc.sync.dma_start(bz_sb[:], bz_d[:, :])
            if use_bias_y:
                by_sb = wpool.tile([BL, O], F32)
                nc.sync.dma_start(by_sb[:], by_d[:, :])

            # identity for PE transposes (bf16: f32 transpose-mode faults on
            # hw); I64 in both partition halves so the fmap can start at
            # partition 0 or 64 (must match the weights)
            ident = wpool.tile([128, 128], mm_dt)
            make_identity(nc, ident[:])
            nc.vector.tensor_copy(ident[:], ident[:])  # launder Pool dep -> DVE

            # c state, channel(32j+b, n) = 256j + n
            c_sb = state.tile([128, 256], F32)
            nc.gpsimd.memset(c_sb[:], 0.0)

            # j outer, beta inner: the beta-pair shares the stationary lhsT
            # AND tile_position, so strip_dup_ldweights() can drop the
            # second (identical) InstLdweights of each pair post-compile.
            def emit_z_mms(z_tile, chunks, start, stop):
                nck = len(chunks)
                for ci, (lhsT, wtile, coff) in enumerate(chunks):
                    for j in range(4):
                        for beta in range(2):
                            w_lo = coff + 2048 * beta + 512 * j
                            nc.tensor.matmul(
                                z_tile[32 * j : 32 * (j + 1),
                                       512 * beta : 512 * (beta + 1)],
                                mc(lhsT),
                                mc(wtile[:, w_lo : w_lo + 512]),
                                start=(start and ci == 0),
                                stop=(stop and ci == nck - 1),
                                tile_position=(0, 32 * j),
                                skip_group_check=True,
                            )

            # x loaded XB steps per DMA (per-instruction dispatch dominates
            # this runtime, so batch I/O into few big transfers)
            xblk = {}

            def x_chunks(t):
                b0 = t // XB
                if b0 not in xblk:
                    xT_sb = xpool.tile([128, XB * 2 * BL], mm_dt, name="xT_sb")
                    nc.sync.dma_start(
                        xT_sb[:],
                        xb_d[b0 * XB : (b0 + 1) * XB].transpose([1, 0, 2]))
                    # launder the x DMA-queue sem into the DVE sem
                    xr_sb = xpool.tile([128, XB * 2 * BL], mm_dt, name="xr_sb")
                    nc.vector.tensor_copy(xr_sb[:], xT_sb[:])
                    xblk[b0] = xr_sb
                base = (t % XB) * 2 * BL
                return [(xblk[b0][:, base + k * BL : base + (k + 1) * BL],
                         Wxx_sb, k * G4) for k in range(KX)]

            hyT_prev = None
            q_blk = None
            # software pipeline: the x-part of step t+1 is issued during step
            # t (cost-neutral in this dispatch-bound runtime, but keeps the
            # matmul dependency structure simple).
            z_ps = zpsum.tile([128, 1024], F32, name="z_ps")
            emit_z_mms(z_ps, x_chunks(0), start=True, stop=(steps_exec == 1))
            for t in range(steps_exec):
                if t > 0:
                    chunks = [(hyT_prev[:, _hT_off(k) : _hT_off(k) + BL], Wh_sb,
                               k * G4) for k in range(KH)]
                    chunks += [(hyT_prev[:, 256 + k * BL : 256 + (k + 1) * BL],
                                Wxy_sb, k * G4) for k in range(KY)]
                    emit_z_mms(z_ps, chunks, start=False, stop=True)
                if t + 1 < steps_exec:
                    z_next = zpsum.tile([128, 1024], F32, name="z_ps")
                    emit_z_mms(z_next, x_chunks(t + 1), start=True, stop=False)
                else:
                    z_next = None

                # gate math, full-width ops (<=1 PSUM operand per DVE op)
                if use_bias_z:
                    nc.vector.tensor_add(z_ps[:, 0:512], z_ps[:, 0:512],
                                         bz_sb[:, 0:512])
                    nc.vector.tensor_add(z_ps[:, 512:1024], z_ps[:, 512:1024],
                                         bz_sb[:, 512:1024])
                tg_sb = gpool.tile([128, 256], F32, name="tg_sb")
                h_stk = gpool.tile([128, 256], mm_dt, name="h_stk")
                tr_ps = tpsum.tile([128, 320], mm_dt, name="tr_ps")
                nc.scalar.activation(tg_sb[:], z_ps[:, 512:768], AF.Tanh)
                nc.scalar.activation(z_ps[:, 0:512], z_ps[:, 0:512], AF.Sigmoid)
                nc.vector.tensor_mul(tg_sb[:], z_ps[:, 0:256], tg_sb[:])
                nc.vector.tensor_mul(c_sb[:], z_ps[:, 256:512], c_sb[:])
                nc.scalar.activation(z_ps[:, 768:1024], z_ps[:, 768:1024],
                                     AF.Sigmoid)
                nc.vector.tensor_add(c_sb[:], tg_sb[:], c_sb[:])
                nc.scalar.activation(tg_sb[:], c_sb[:], AF.Tanh)
                nc.vector.tensor_mul(h_stk[:], z_ps[:, 768:1024], tg_sb[:])
                nc.tensor.transpose(tr_ps[:, 0:128], h_stk[:, 0:128], ident[:])
                nc.tensor.transpose(tr_ps[:, 128:256], h_stk[:, 128:256],
                                    ident[:])
                # hT needed now for the y-matmuls: copy it out of PSUM; the
                # yT columns (256:320) are appended after the y feedback so
                # one [128, 320] tile carries both transposed states
                hyT_sb = hpool.tile([128, 320], mm_dt, name="hyT_sb", bufs=2)
                nc.vector.tensor_copy(hyT_sb[:, 0:256], tr_ps[:, 0:256])

                # y = tanh(h @ Wd + bd)
                y_ps = ypsum.tile([BL, O], F32, name="y_ps")
                if FUNNEL:
                    nc.vector.tensor_copy(y_ps[:], Wxx_sb[0:BL, 0:256])
                for k in range(KH):
                    nc.tensor.matmul(
                        y_ps[:],
                        mc(hyT_sb[:, _hT_off(k) : _hT_off(k) + BL]),
                        mc(Wd_sb[:, k * O : (k + 1) * O]),
                        start=(k == 0),
                        stop=(k == KH - 1),
                    )
                if use_bias_y:
                    nc.vector.tensor_add(y_ps[:], y_ps[:], by_sb[:])
                y_bf = ypool.tile([BL, O], mm_dt, name="y_bf", bufs=2)
                nc.scalar.activation(y_bf[:], y_ps[:], AF.Tanh)
                # offset-uint8 quantize on ACT (rne conversion); XB steps
                # buffered in SBUF then stored with one DMA
                if t % XB == 0:
                    q_blk = ypool.tile([BL, XB * O], U8, name="q_blk", bufs=2)
                nc.scalar.activation(q_blk[:, (t % XB) * O : (t % XB + 1) * O],
                                     y_bf[:], AF.Copy, scale=127.0, bias=127.5)
                if t % XB == XB - 1 or t == steps_exec - 1:
                    t0 = t - (t % XB)
                    nc.sync.dma_start(ys_d[:, t0 : t + 1, :],
                                      q_blk[:, 0 : (t % XB + 1) * O])
                # DVE-launder for the PE transposes (matmul deps must
                # resolve to a single DVE sem value)
                y_tr = ypool.tile([BL, O], mm_dt, name="y_tr")
                nc.vector.tensor_copy(y_tr[:], y_bf[:])

                # y -> yT via 2 PE transposes
                for q in range(2):
                    nc.tensor.transpose(
                        tr_ps[:, 256 + 32 * q : 256 + 32 * (q + 1)],
                        y_tr[0:BL, 128 * q : 128 * (q + 1)],
                        ident[0:32, 0:32],
                    )
                nc.vector.tensor_copy(hyT_sb[:, 256:320], tr_ps[:, 256:320])

                hyT_prev = hyT_sb
                z_ps = z_next

    nc.compile()
    strip_dup_ldweights(nc)
    return nc


def strip_dup_ldweights(nc):
    """Drop an InstLdweights identical to the immediately-preceding one.

    Each nc.tensor.matmul lowers to an Ldweights+Matmult pair, and this
    runtime charges ~dispatch cost per instruction, so the beta-pair's
    second (bit-identical stationary data, same tile_position) reload is
    pure overhead. Matmult does not alter the loaded array, so dropping an
    exact consecutive duplicate is semantics-preserving. Instructions that
    carry semaphore waits/updates are kept (their sync actions must fire).
    """
    removed = 0
    for bb in nc.m.functions[0].blocks:
        keep = []
        last_sig = None
        for ins in bb.instructions:
            if isinstance(ins, mybir.InstLdweights):
                sig = (repr(ins.ins), repr(ins.tile_position),
                       repr(ins.is_transpose), repr(ins.perf_mode))
                si = ins.sync_info
                clean = si is None or (len(si.on_wait) == 0
                                       and len(si.on_update) == 0)
                if clean and sig == last_sig:
                    removed += 1
                    continue
                last_sig = sig
            keep.append(ins)
        if len(keep) != len(bb.instructions):
            bb.instructions[:] = keep
    return removed


def prep_inputs(x, Wx, Wh, b, Wd, bd, T_steps: int = T):
    """Host-side shard + relayout. Returns (in_maps, use_bias_z, use_bias_y)."""
    x = np.asarray(x, dtype=np.float32)[:, :T_steps, :]
    Wx = np.asarray(Wx, dtype=np.float32)
    Wh = np.asarray(Wh, dtype=np.float32)
    b = np.asarray(b, dtype=np.float32)
    Wd = np.asarray(Wd, dtype=np.float32)
    bd = np.asarray(bd, dtype=np.float32)

    import ml_dtypes

    perm = gate_perm()
    Wxp = np.ascontiguousarray(Wx[:, perm]).astype(ml_dtypes.bfloat16)
    Whp = np.ascontiguousarray(Wh[:, perm]).astype(ml_dtypes.bfloat16)
    # blob rows: Wxx (0:D) | Wxy (D:D+O) | Wh (D+O:WROWS) — each core ships
    # 1/8 of the rows, the kernel AllGathers the full blob device-side
    wblob = np.concatenate([Wxp, Whp], axis=0)  # [WROWS, 4H]
    Wd = Wd.astype(ml_dtypes.bfloat16)

    use_bias_z = bool(np.any(b))
    use_bias_y = bool(np.any(bd))
    shared = {}
    if use_bias_z:
        bp = b[perm]
        bz = np.empty((128, 1024), dtype=np.float32)
        for j in range(4):
            for beta in range(2):
                bz[32 * j : 32 * (j + 1), 512 * beta : 512 * (beta + 1)] = bp[
                    2048 * beta + 512 * j : 2048 * beta + 512 * j + 512][None, :]
        shared["bz"] = bz
    if use_bias_y:
        shared["by"] = np.broadcast_to(bd, (BL, O)).copy()

    in_maps = []
    for c in range(NCORES):
        xc = x[c * BL : (c + 1) * BL]                      # [BL, T, D]
        xT = xc.transpose(1, 2, 0)                         # [T, D, BL]
        xT = xT.reshape(T_steps, 2, 128, BL).transpose(0, 2, 1, 3)
        xT = np.ascontiguousarray(
            xT.reshape(T_steps, 128, 2 * BL)).astype(ml_dtypes.bfloat16)
        # single packed input tensor per core: each transfer buffer pays a
        # large fixed cost on the axon tunnel (0.5MB standalone moved at
        # ~7MB/s), so x + weight shards ship as one buffer. The weight
        # shards are raw row-major bytes reinterpreted as "fake steps".
        xb = np.concatenate([
            xT,
            wblob[c * WSH : (c + 1) * WSH].reshape(WFS, 128, 2 * BL),
            Wd[c * WDSH : (c + 1) * WDSH].reshape(DFS, 128, 2 * BL),
        ], axis=0)
        in_maps.append({"xb": np.ascontiguousarray(xb), **shared})
    return in_maps, use_bias_z, use_bias_y


def postprocess(res):
    """Gather per-core offset-uint8 ys and decode to fp32 [B, T, O]."""
    q = np.concatenate([res.results[c]["ys"] for c in range(NCORES)], axis=0)
    return (q.astype(np.float32) - 127.5) * (1.0 / 127.0)


def kernel(x, Wx, Wh, b, Wd, bd):
    from concourse.bass_utils import run_bass_kernel_spmd

    in_maps, ubz, uby = prep_inputs(x, Wx, Wh, b, Wd, bd, T)
    nc = build_nc(T, ubz, uby)
    res = run_bass_kernel_spmd(nc, in_maps, list(range(NCORES)))
    return postprocess(res)

